# revision 34
# baseline (speedup 1.0000x reference)
"""GCN layer kernel for 8 Trainium2 NeuronCores.

Computes: out = relu(dinv[:,None] * ((adj+I).T @ (dinv[:,None] * (x@W))) + b)
where dinv = rsqrt(colsum(adj) + 1).

Strategy (mode "pk"): shard adj by COLUMNS across the 8 cores -> no
collectives. Two adjacency entries (adjacent output columns, values {0,1,2}
with the self-loop folded) ship per HBM byte: byte = 4*a_even + a_odd.
Bytes 0..15 read as fp8e4 denormals are exactly k*2^-9, so the packed byte
is VALUE-LINEAR in both entries and can be fed to the PE directly.  Per
k-pair the PE runs two DoubleRow passes sharing one stationary z load:
pass 1 on the packed tile accumulates 2^-9*(4*out_e + out_o), pass 2 on the
DVE-extracted odd plane (one bitwise_and with 0x03030303 per tile, via a
uint32 bitcast) accumulates 2^-9*out_o.  The post-pass unmixes, applies
dinv (scales pre-folded on host), bias and relu.  HBM traffic for the
adjacency halves to 16.7MB/core; the PE stays at its moving-wire floor
(2 fp8 entries/partition/cycle, DoubleRow).

Precision: z = dinv*(x@W) is split into z_hi = fp8(z) and
z_lo = fp8(32*(z - z_hi)), occupying stationary PE columns 0-63 / 64-127 --
the otherwise-idle half of the array computes the correction term for free
(final rel err ~1.6e-3, same as a bf16 kernel).

Perf notes: bass emits one weight (re)load per matmul which serializes with
the MMs; _dedup_ldweights drops the redundant reloads (4 matmuls per k-pair
share one stationary).  PSUM is double-buffered so back-to-back passes
overlap the drain/post of the previous one.
"""

import sys

import numpy as np

if "/opt/trn_rl_repo" not in sys.path:
    sys.path.insert(0, "/opt/trn_rl_repo")

import ml_dtypes

N = 16384
F = 64
NCORES = 8
NB = N // NCORES  # 2048 columns (= output rows) per core
P = 128
KT = N // P  # 128 k-tiles of 128 source rows each
KP = KT // 2  # 64 DoubleRow k-pair steps
MM_N = 512  # moving-operand output cols per matmul (one PSUM bank of f32)
DMA_BATCH = 2  # k-tiles per dma_start (must be even for dr modes)
APOOL_BUFS = 8  # in-flight A-tile slots (prefetch depth)
PSUM_BUFS = 2  # PSUM accumulator sets (2 = overlap reps)
GP_UNPACK = False  # alternate the bitwise unpack between DVE and GpSimd
ALT_DMA = False  # alternate A-tile DMAs between the SP and ACT HWDGE rings
LO_SCALE = 32.0  # z_lo quantization upscale
DEDUP_LDW = True  # drop redundant back-to-back identical weight reloads
MODE = "pksw"  # variant kernel() uses

_BASS_CACHE: dict = {}


def _build_bass(reps: int = 1, mode: str = None):
    if mode is None:
        mode = MODE
    key = (reps, mode, DMA_BATCH, APOOL_BUFS, ALT_DMA, PSUM_BUFS, GP_UNPACK)
    if key in _BASS_CACHE:
        return _BASS_CACHE[key]
    if mode.startswith("pk"):
        nc = _build_bass_pk(reps, mode)
    elif mode.startswith("dr"):
        nc = _build_bass_dr(reps, mode)
    else:
        nc = _build_bass_pair(reps, mode)
    _BASS_CACHE[key] = nc
    return nc


def _build_bass_pk(reps: int, mode: str):
    """Packed-pair DoubleRow kernel: adjacency ships 2 output columns per
    byte (byte = 4*a_even + a_odd, read as fp8 denormal k*2^-9, exactly
    linear). DVE extracts the odd plane with one bitwise_and per tile; two
    DoubleRow passes (packed, odd) share the same stationary z per k-pair.
    HBM traffic for the adjacency halves to 16.7MB/core.
    mode: "pk" | "pkmm" (resident tile) | "pkdma" (loads only)."""
    import concourse.mybir as mybir
    import concourse.tile as tile
    from concourse import bacc

    nc = bacc.Bacc("TRN2", target_bir_lowering=False, debug=False,
                   num_devices=NCORES)

    kb = DMA_BATCH
    assert kb % 2 == 0 and KT % kb == 0
    G = KT // kb
    NB2 = NB // 2
    fp8 = mybir.dt.float8e4
    f32 = mybir.dt.float32
    sw = "sw" in mode
    mode = mode.replace("sw", "")
    DR = (mybir.MatmulPerfMode.DoubleRowSwInterleave if sw
          else mybir.MatmulPerfMode.DoubleRow)

    a_in = nc.dram_tensor("a", [N, NB2], fp8, kind="ExternalInput")
    z_in = nc.dram_tensor("z", [P, KP * 2 * P], fp8, kind="ExternalInput")
    # d[0:64, 0:NB2]=128*dinv_e, d[64:128, 0:NB2]=same/32,
    # d[0:64, NB2:]=512*dinv_o, d[64:128, NB2:]=same/32
    d_in = nc.dram_tensor("dinv", [2 * F, 2 * NB2], f32, kind="ExternalInput")
    b_in = nc.dram_tensor("bvec", [F, 1], f32, kind="ExternalInput")
    # rows 0-63: even output cols, rows 64-127: odd output cols
    o_out = nc.dram_tensor("o", [2 * F, NB2], f32, kind="ExternalOutput")

    a_tiles = a_in.ap().rearrange("(g t p) i -> g p t i", t=kb, p=P)

    with tile.TileContext(nc) as tc:
        with (
            tc.tile_pool(name="singles", bufs=1) as singles,
            tc.tile_pool(name="apool", bufs=APOOL_BUFS) as apool,
            tc.tile_pool(name="bpool", bufs=APOOL_BUFS) as bpool,
            tc.tile_pool(name="psum", bufs=PSUM_BUFS, space="PSUM") as psum_pool,
            tc.tile_pool(name="post", bufs=PSUM_BUFS) as post_pool,
        ):
            z_sb = singles.tile([P, KP * 128, 2] if sw else [P, KP * 2, P],
                                fp8)
            nc.scalar.dma_start(z_sb[:], z_in.ap())
            d_sb = singles.tile([2 * F, 2 * NB2], f32, tag="d_sb")
            nc.scalar.dma_start(d_sb[:], d_in.ap())
            b_sb = singles.tile([F, 1], f32, tag="b_sb")
            nc.scalar.dma_start(b_sb[:], b_in.ap())

            mm_tile = mm_btile = None
            if mode == "pkmm":
                mm_tile = singles.tile([P, kb, NB2], fp8, tag="mm_tile")
                nc.sync.dma_start(mm_tile[:], a_tiles[0])
                mm_btile = singles.tile([P, kb, NB2], fp8, tag="mm_btile")
                nc.vector.tensor_scalar(
                    mm_btile[:].bitcast(mybir.dt.uint32),
                    mm_tile[:].bitcast(mybir.dt.uint32),
                    0x03030303, None, mybir.AluOpType.bitwise_and)

            relu = mybir.ActivationFunctionType.Relu

            for _rep in range(reps):
                ps1 = psum_pool.tile([P, NB2], f32)
                ps2 = psum_pool.tile([P, NB2], f32, tag="ps2")
                oe_sb = post_pool.tile([F, NB2], f32, tag="oe_sb")
                oo_sb = post_pool.tile([F, NB2], f32, tag="oo_sb")
                c2_sb = post_pool.tile([P, NB2], f32, tag="c2_sb")
                u_sb = post_pool.tile([P, NB2], f32, tag="u_sb")
                t2_sb = post_pool.tile([F, NB2], f32, tag="t2_sb")

                for g in range(G):
                    if mode == "pkmm":
                        at, bt = mm_tile, mm_btile
                    else:
                        at = apool.tile([P, kb, NB2], fp8)
                        eng = nc.gpsimd if (ALT_DMA and g % 2) else nc.sync
                        eng.dma_start(at[:], a_tiles[g])
                        bt = bpool.tile([P, kb, NB2], fp8)
                        ueng = nc.gpsimd if (GP_UNPACK and g % 2) else nc.vector
                        ueng.tensor_scalar(
                            bt[:].bitcast(mybir.dt.uint32),
                            at[:].bitcast(mybir.dt.uint32),
                            0x03030303, None, mybir.AluOpType.bitwise_and)
                    if mode == "pkdma":
                        continue
                    for jj in range(kb // 2):
                        kp = g * (kb // 2) + jj
                        zk = (z_sb[:, kp * 128:(kp + 1) * 128, :] if sw
                              else z_sb[:, 2 * kp:2 * kp + 2, :])
                        for nn in range(NB2 // MM_N):
                            sf = slice(nn * MM_N, (nn + 1) * MM_N)
                            nc.tensor.matmul(
                                ps1[:, sf], lhsT=zk,
                                rhs=at[:, 2 * jj:2 * jj + 2, sf],
                                start=(kp == 0), stop=(kp == KP - 1),
                                perf_mode=DR)
                        for nn in range(NB2 // MM_N):
                            sf = slice(nn * MM_N, (nn + 1) * MM_N)
                            nc.tensor.matmul(
                                ps2[:, sf], lhsT=zk,
                                rhs=bt[:, 2 * jj:2 * jj + 2, sf],
                                start=(kp == 0), stop=(kp == KP - 1),
                                perf_mode=DR)

                if mode == "pkdma":
                    nc.vector.tensor_copy(oe_sb[:, :F], d_sb[:F, :F])
                    nc.sync.dma_start(o_out.ap()[:F, :], oe_sb[:])
                else:
                    for nn in range(NB2 // MM_N):
                        sf = slice(nn * MM_N, (nn + 1) * MM_N)
                        so = slice(NB2 + nn * MM_N, NB2 + (nn + 1) * MM_N)
                        # stage C2 to SBUF on the scalar engine (DVE may
                        # read at most one PSUM operand per instruction)
                        nc.scalar.copy(c2_sb[:, sf], ps2[:, sf])
                        # u = C1 - C2 (both hi and lo halves at once)
                        nc.vector.tensor_sub(u_sb[:, sf], ps1[:, sf],
                                             c2_sb[:, sf])
                        # out_e = relu((u_hi*De + u_lo*De32) + b)
                        nc.vector.tensor_mul(oe_sb[:, sf], u_sb[:F, sf],
                                             d_sb[:F, sf])
                        nc.vector.tensor_mul(t2_sb[:, sf], u_sb[F:, sf],
                                             d_sb[F:, sf])
                        nc.vector.tensor_add(oe_sb[:, sf], oe_sb[:, sf],
                                             t2_sb[:, sf])
                        nc.scalar.activation(oe_sb[:, sf], oe_sb[:, sf],
                                             relu, bias=b_sb[:], scale=1.0)
                        nc.scalar.dma_start(o_out.ap()[:F, sf], oe_sb[:, sf])
                        # out_o = relu((C2_hi*Do + C2_lo*Do32) + b)
                        nc.vector.tensor_mul(oo_sb[:, sf], c2_sb[:F, sf],
                                             d_sb[:F, so])
                        nc.vector.tensor_mul(t2_sb[:, sf], c2_sb[F:, sf],
                                             d_sb[F:, so])
                        nc.vector.tensor_add(oo_sb[:, sf], oo_sb[:, sf],
                                             t2_sb[:, sf])
                        nc.scalar.activation(oo_sb[:, sf], oo_sb[:, sf],
                                             relu, bias=b_sb[:], scale=1.0)
                        nc.scalar.dma_start(o_out.ap()[F:, sf], oo_sb[:, sf])

    if DEDUP_LDW:
        _dedup_ldweights(nc)
    nc.compile()
    return nc


def _dedup_ldweights(nc):
    """Remove InstLdweights whose weights-AP is identical to the previous
    (kept) InstLdweights on the same engine with only InstMatmult between.
    The chunk loop reuses one stationary operand for 4 matmuls; bass emits a
    reload before each, serializing LDW with MM on the PE. Matmuls carry
    ldweights=False and nothing references the removed loads by name, so
    dropping them is safe; PE program order keeps the kept load first."""
    for fn in nc.m.functions:
        for blk in fn.blocks:
            out, last_sig = [], None
            for inst in blk.instructions:
                tn = type(inst).__name__
                if tn == "InstLdweights":
                    sig = (repr(inst.ins[0]),
                           frozenset(inst.sync_dependency_names() or []),
                           frozenset(inst.nosync_dependency_names() or []))
                    if sig == last_sig:
                        continue
                    last_sig = sig
                elif tn != "InstMatmult":
                    last_sig = None
                out.append(inst)
            blk.instructions = out


def _build_bass_dr(reps: int, mode: str):
    """DoubleRow fp8 kernel. mode: "dr" | "drmm" (matmuls from one resident
    tile, 1/64th DMA) | "drdma" (loads only, no matmul)."""
    import concourse.mybir as mybir
    import concourse.tile as tile
    from concourse import bacc

    nc = bacc.Bacc("TRN2", target_bir_lowering=False, debug=False,
                   num_devices=NCORES)

    kb = DMA_BATCH
    assert kb % 2 == 0 and KT % kb == 0
    G = KT // kb  # DMA groups
    fp8 = mybir.dt.float8e4
    DR = (mybir.MatmulPerfMode.DoubleRowSwInterleave if "sw" in mode
          else mybir.MatmulPerfMode.DoubleRow)
    mode = mode.replace("sw", "")  # behavior keyed on base mode

    a_in = nc.dram_tensor("a", [N, NB], fp8, kind="ExternalInput")
    z_in = nc.dram_tensor("z", [P, KP * 2 * P], fp8, kind="ExternalInput")
    # rows 0-63: dinv (hi), rows 64-127: dinv/LO_SCALE (lo)
    d_in = nc.dram_tensor("dinv", [2 * F, NB], mybir.dt.float32,
                          kind="ExternalInput")
    b_in = nc.dram_tensor("bvec", [F, 1], mybir.dt.float32,
                          kind="ExternalInput")
    o_out = nc.dram_tensor("o", [F, NB], mybir.dt.float32,
                           kind="ExternalOutput")

    # [G, 128, kb, NB]: row g*kb*128 + t*128 + p -> a_tiles[g][p, t, :]
    a_tiles = a_in.ap().rearrange("(g t p) i -> g p t i", t=kb, p=P)

    with tile.TileContext(nc) as tc:
        with (
            tc.tile_pool(name="singles", bufs=1) as singles,
            tc.tile_pool(name="apool", bufs=APOOL_BUFS) as apool,
            tc.tile_pool(name="psum", bufs=1, space="PSUM") as psum_pool,
        ):
            # z_sb[p, kp*2+j, m]: m 0-63 = z_hi feature, 64-127 = z_lo
            z_sb = singles.tile([P, KP * 2, P], fp8)
            nc.sync.dma_start(z_sb[:], z_in.ap())
            d_sb = singles.tile([2 * F, NB], mybir.dt.float32, tag="d_sb")
            nc.sync.dma_start(d_sb[:], d_in.ap())
            b_sb = singles.tile([F, 1], mybir.dt.float32, tag="b_sb")
            nc.sync.dma_start(b_sb[:], b_in.ap())
            out_sb = singles.tile([F, NB], mybir.dt.float32, tag="out_sb")
            tmp_sb = singles.tile([F, NB], mybir.dt.float32, tag="tmp_sb")

            mm_tile = None
            if mode == "drmm":
                mm_tile = singles.tile([P, kb, NB], fp8, tag="mm_tile")
                nc.sync.dma_start(mm_tile[:], a_tiles[0])
            mm_tiles = None
            if mode == "drmm2":
                mm_tiles = []
                for i in range(8):
                    t = singles.tile([P, kb, NB], fp8, tag=f"mm_tile{i}")
                    nc.sync.dma_start(t[:], a_tiles[i])
                    mm_tiles.append(t)

            relu = mybir.ActivationFunctionType.Relu

            for _rep in range(reps):
                ps = psum_pool.tile([P, NB], mybir.dt.float32)

                for g in range(G):
                    if mode == "drmm":
                        at = mm_tile
                    elif mode == "drmm2":
                        at = mm_tiles[g % 8]
                    else:
                        at = apool.tile([P, kb, NB], fp8)
                        eng = nc.scalar if (ALT_DMA and g % 2) else nc.sync
                        eng.dma_start(at[:], a_tiles[g])
                    if mode == "drdma":
                        continue
                    for jj in range(kb // 2):
                        kp = g * (kb // 2) + jj
                        zk = z_sb[:, 2 * kp:2 * kp + 2, :]
                        for nn in range(NB // MM_N):
                            sf = slice(nn * MM_N, (nn + 1) * MM_N)
                            nc.tensor.matmul(
                                ps[:, sf],
                                lhsT=zk,
                                rhs=at[:, 2 * jj:2 * jj + 2, sf],
                                start=(kp == 0),
                                stop=(kp == KP - 1),
                                perf_mode=DR,
                            )

                if mode == "drdma":
                    nc.vector.tensor_copy(out_sb[:, :F], d_sb[:F, :F])
                    nc.sync.dma_start(o_out.ap(), out_sb[:])
                else:
                    for nn in range(NB // MM_N):
                        sf = slice(nn * MM_N, (nn + 1) * MM_N)
                        # out = relu((ps_hi*dinv + ps_lo*dinv/LO_SCALE) + b)
                        nc.vector.tensor_mul(tmp_sb[:, sf], ps[F:2 * F, sf],
                                             d_sb[F:2 * F, sf])
                        nc.vector.tensor_mul(out_sb[:, sf], ps[:F, sf],
                                             d_sb[:F, sf])
                        nc.vector.tensor_add(out_sb[:, sf], out_sb[:, sf],
                                             tmp_sb[:, sf])
                        nc.scalar.activation(out_sb[:, sf], out_sb[:, sf],
                                             relu, bias=b_sb[:], scale=1.0)
                        nc.sync.dma_start(o_out.ap()[:, sf], out_sb[:, sf])

    if DEDUP_LDW:
        _dedup_ldweights(nc)
    nc.compile()
    return nc


def _build_bass_pair(reps: int, mode: str):
    """Legacy fp8+col-pair-tiling kernel. mode: "fp8pair" | "dma8" | "mm8"."""
    import concourse.mybir as mybir
    import concourse.tile as tile
    from concourse import bacc

    nc = bacc.Bacc("TRN2", target_bir_lowering=False, debug=False,
                   num_devices=NCORES)

    fp8 = mode in ("fp8", "fp8pair", "dma8", "mm8")
    pair = mode in ("pair", "fp8pair")
    a_dt = mybir.dt.float8e4 if fp8 else mybir.dt.bfloat16
    b_p = 2 * F if pair else F
    a_in = nc.dram_tensor("a", [N, NB], a_dt, kind="ExternalInput")
    z_in = nc.dram_tensor("z", [P, KT * F], mybir.dt.bfloat16,
                          kind="ExternalInput")
    b_in = nc.dram_tensor("bvec", [b_p, 1], mybir.dt.float32,
                          kind="ExternalInput")
    if fp8:
        d_in = nc.dram_tensor("dinv", [b_p, NB], mybir.dt.float32,
                              kind="ExternalInput")
    o_out = nc.dram_tensor("o", [F, NB], mybir.dt.float32,
                           kind="ExternalOutput")

    kb = DMA_BATCH
    a_tiles = a_in.ap().rearrange("(g t p) i -> g p t i", t=kb, p=P)

    with tile.TileContext(nc) as tc:
        with (
            tc.tile_pool(name="singles", bufs=1) as singles,
            tc.tile_pool(name="apool", bufs=APOOL_BUFS) as apool,
            tc.tile_pool(name="psum", bufs=1, space="PSUM") as psum_pool,
        ):
            z_sb = singles.tile([P, KT * F], mybir.dt.bfloat16)
            nc.sync.dma_start(z_sb[:], z_in.ap())
            b_sb = singles.tile([b_p, 1], mybir.dt.float32)
            nc.sync.dma_start(b_sb[:], b_in.ap())
            d_sb = None
            if fp8:
                d_sb = singles.tile([b_p, NB], mybir.dt.float32, tag="d_sb")
                nc.sync.dma_start(d_sb[:], d_in.ap())

            mm_tile = None
            if mode in ("mm", "mm8"):
                mm_tile = singles.tile([P, kb, NB], a_dt, tag="mm_tile")
                nc.sync.dma_start(mm_tile[:], a_tiles[0])

            for _rep in range(reps):
                ps = psum_pool.tile([b_p, NB], mybir.dt.float32)

                for g in range(KT // kb):
                    if mode in ("mm", "mm8"):
                        at = mm_tile
                    else:
                        at = apool.tile([P, kb, NB], a_dt)
                        eng = nc.scalar if (ALT_DMA and g % 2) else nc.sync
                        eng.dma_start(at[:], a_tiles[g])
                    if mode in ("dma", "dma8"):
                        continue
                    for t in range(kb):
                        kt = g * kb + t
                        zk = z_sb[:, kt * F:(kt + 1) * F]
                        if pair:
                            for nn in range(NB // MM_N):
                                h = nn % 2
                                nc.tensor.matmul(
                                    ps[h * F:(h + 1) * F,
                                       nn * MM_N:(nn + 1) * MM_N],
                                    lhsT=zk,
                                    rhs=at[:, t, nn * MM_N:(nn + 1) * MM_N],
                                    start=(kt == 0),
                                    stop=(kt == KT - 1),
                                    tile_position=(0, h * F),
                                )
                        else:
                            for nn in range(NB // MM_N):
                                nc.tensor.matmul(
                                    ps[:, nn * MM_N:(nn + 1) * MM_N],
                                    lhsT=zk,
                                    rhs=at[:, t, nn * MM_N:(nn + 1) * MM_N],
                                    start=(kt == 0),
                                    stop=(kt == KT - 1),
                                )

                out_sb = singles.tile([b_p, NB], mybir.dt.float32,
                                      tag="out_sb")
                relu = mybir.ActivationFunctionType.Relu
                if mode in ("dma", "dma8"):
                    nc.vector.tensor_copy(out_sb[:F, :F], z_sb[:F, :F])
                    nc.sync.dma_start(o_out.ap(), out_sb[:F, :])
                elif pair:
                    for nn in range(NB // MM_N):
                        h = nn % 2
                        sp = slice(h * F, (h + 1) * F)
                        sf = slice(nn * MM_N, (nn + 1) * MM_N)
                        if fp8:
                            nc.vector.tensor_mul(out_sb[sp, sf], ps[sp, sf],
                                                 d_sb[sp, sf])
                            nc.scalar.activation(out_sb[sp, sf],
                                                 out_sb[sp, sf], relu,
                                                 bias=b_sb[sp], scale=1.0)
                        else:
                            nc.scalar.activation(out_sb[sp, sf], ps[sp, sf],
                                                 relu, bias=b_sb[sp],
                                                 scale=1.0)
                        nc.sync.dma_start(o_out.ap()[:, sf], out_sb[sp, sf])
                elif fp8:
                    nc.vector.tensor_mul(out_sb[:], ps[:], d_sb[:])
                    nc.scalar.activation(out_sb[:], out_sb[:], relu,
                                         bias=b_sb[:], scale=1.0)
                    nc.sync.dma_start(o_out.ap(), out_sb[:])
                else:
                    nc.scalar.activation(out_sb[:], ps[:], relu,
                                         bias=b_sb[:], scale=1.0)
                    nc.sync.dma_start(o_out.ap(), out_sb[:])

    nc.compile()
    return nc


def _host_prep(x, adj, W, b, mode=None):
    """Host-side sharding/preprocessing -> per-core input maps."""
    if mode is None:
        mode = MODE
    x = np.asarray(x, dtype=np.float32)
    adj = np.asarray(adj, dtype=np.float32)
    W = np.asarray(W, dtype=np.float32)
    b = np.asarray(b, dtype=np.float32)

    deg = adj.sum(axis=0) + 1.0
    dinv = np.where(deg > 0, 1.0 / np.sqrt(deg), 0.0).astype(np.float32)
    z = (dinv[:, None] * (x @ W)).astype(np.float32)  # [N, F]

    if mode.startswith("pk"):
        return _host_prep_pk(z, adj, b, dinv, sw="sw" in mode)
    if mode.startswith("dr"):
        return _host_prep_dr(z, adj, b, dinv)
    return _host_prep_pair(z, adj, b, dinv, mode)


def _host_prep_pk(z, adj, b, dinv, sw=False):
    NB2 = NB // 2
    z_hi8 = z.astype(ml_dtypes.float8_e4m3)
    z_lo8 = ((z - z_hi8.astype(np.float32)) * LO_SCALE).astype(
        ml_dtypes.float8_e4m3)
    hi = z_hi8.reshape(KP, 2, P, F).transpose(2, 0, 1, 3)
    lo = z_lo8.reshape(KP, 2, P, F).transpose(2, 0, 1, 3)
    w = np.concatenate([hi, lo], axis=3)  # [P, KP, 2, 128]
    if sw:
        # SwInterleave storage: flat[2c + j] = w[:, :, j, 127 - c]
        z_dev = np.ascontiguousarray(
            w[:, :, :, ::-1].transpose(0, 1, 3, 2).reshape(P, KP * 2 * P))
    else:
        z_dev = np.ascontiguousarray(w.reshape(P, KP * 2 * P))
    b_dev = np.ascontiguousarray(b.reshape(F, 1))

    in_maps = []
    idx = np.arange(NB)
    for c in range(NCORES):
        cs = c * NB
        blk = adj[:, cs:cs + NB].copy()
        blk[cs + idx, idx] += 1.0  # fold self-loop (+I)
        packed = (4.0 * blk[:, 0::2] + blk[:, 1::2]).astype(np.uint8)
        dc = dinv[cs:cs + NB]
        d_dev = np.empty((2 * F, 2 * NB2), np.float32)
        d_dev[:F, :NB2] = 128.0 * dc[0::2]
        d_dev[F:, :NB2] = 128.0 * dc[0::2] / LO_SCALE
        d_dev[:F, NB2:] = 512.0 * dc[1::2]
        d_dev[F:, NB2:] = 512.0 * dc[1::2] / LO_SCALE
        m = {
            "a": packed.view(ml_dtypes.float8_e4m3),
            "z": z_dev,
            "dinv": d_dev,
            "bvec": b_dev,
        }
        in_maps.append(m)
    return in_maps


def _host_prep_dr(z, adj, b, dinv):
    z_hi8 = z.astype(ml_dtypes.float8_e4m3)
    z_lo8 = ((z - z_hi8.astype(np.float32)) * LO_SCALE).astype(
        ml_dtypes.float8_e4m3)
    # [P, KP, 2, 128]: row kp*256 + j*128 + p -> [p, kp, j, :]
    hi = z_hi8.reshape(KP, 2, P, F).transpose(2, 0, 1, 3)
    lo = z_lo8.reshape(KP, 2, P, F).transpose(2, 0, 1, 3)
    z_dev = np.ascontiguousarray(
        np.concatenate([hi, lo], axis=3).reshape(P, KP * 2 * P))

    b_dev = np.ascontiguousarray(b.reshape(F, 1))

    in_maps = []
    idx = np.arange(NB)
    for c in range(NCORES):
        cs = c * NB
        blk = adj[:, cs:cs + NB].copy()
        blk[cs + idx, idx] += 1.0  # fold self-loop (+I); {0,1,2} exact
        dc = dinv[cs:cs + NB]
        d_dev = np.empty((2 * F, NB), np.float32)
        d_dev[:F] = dc
        d_dev[F:] = dc / LO_SCALE
        m = {
            "a": blk.astype(ml_dtypes.float8_e4m3),
            "z": z_dev,
            "dinv": d_dev,
            "bvec": b_dev,
        }
        in_maps.append(m)
    return in_maps


def _host_prep_pair(z, adj, b, dinv, mode):
    fp8 = mode in ("fp8", "fp8pair", "dma8", "mm8")
    pair = mode in ("pair", "fp8pair")
    # k-major layout: z_sb[p, kt*F + f] = z[kt*128 + p, f]
    z_dev = np.ascontiguousarray(
        z.reshape(KT, P, F).transpose(1, 0, 2).reshape(P, KT * F)
    ).astype(ml_dtypes.bfloat16)

    if pair:
        b_dev = np.ascontiguousarray(
            np.concatenate([b, b]).reshape(2 * F, 1))
    else:
        b_dev = np.ascontiguousarray(b.reshape(F, 1))

    def _pair_dinv(dc):
        d = np.zeros((2 * F, NB), np.float32)
        for nn in range(NB // MM_N):
            h = nn % 2
            d[h * F:(h + 1) * F, nn * MM_N:(nn + 1) * MM_N] = \
                dc[nn * MM_N:(nn + 1) * MM_N]
        return d

    in_maps = []
    idx = np.arange(NB)
    for c in range(NCORES):
        cs = c * NB
        if fp8:
            blk = adj[:, cs:cs + NB].copy()
            blk[cs + idx, idx] += 1.0
            dc = dinv[cs:cs + NB]
            m = {
                "a": blk.astype(ml_dtypes.float8_e4m3),
                "z": z_dev,
                "bvec": b_dev,
                "dinv": (_pair_dinv(dc) if pair else np.ascontiguousarray(
                    np.broadcast_to(dc, (F, NB)))),
            }
        else:
            blk = adj[:, cs:cs + NB] * dinv[cs:cs + NB][None, :]
            blk[cs + idx, idx] += dinv[cs + idx]
            m = {
                "a": blk.astype(ml_dtypes.bfloat16),
                "z": z_dev,
                "bvec": b_dev,
            }
        in_maps.append(m)
    return in_maps


def _assemble(results, mode=None):
    """Device outputs -> full [N, F] output."""
    if mode is None:
        mode = MODE
    out = np.empty((N, F), dtype=np.float32)
    if mode.startswith("pk"):
        for c in range(NCORES):
            r = results[c]["o"]  # [128, NB/2]: rows 0-63 even, 64-127 odd
            oe, oo = r[:F].T, r[F:].T
            out[c * NB:(c + 1) * NB:2, :] = oe
            out[c * NB + 1:(c + 1) * NB:2, :] = oo
        return out
    for c in range(NCORES):
        out[c * NB:(c + 1) * NB, :] = results[c]["o"].T
    return out


def kernel(x, adj, W, b):
    from concourse import bass_utils

    nc = _build_bass(mode=MODE)
    in_maps = _host_prep(x, adj, W, b, mode=MODE)
    res = bass_utils.run_bass_kernel_spmd(nc, in_maps,
                                          core_ids=list(range(NCORES)))
    return _assemble(res.results, mode=MODE)
